# revision 1
# baseline (speedup 1.0000x reference)
import sys

sys.path.insert(0, "/opt/trn_rl_repo")
import numpy as np
from contextlib import ExitStack

from concourse import bacc
import concourse.tile as tile
from concourse import mybir
from concourse.bass_utils import run_bass_kernel_spmd

fp32 = mybir.dt.float32
bf16 = mybir.dt.bfloat16
Exp = mybir.ActivationFunctionType.Exp

B, S, HID = 4, 2048, 1024
H, DK = 16, 64
SK = 1152          # compacted+padded key count (max kept keys = 1036)
SKT = SK // 128    # 9 sk tiles
NPAIR = 4          # head pairs per core (8 heads = half the 16)

# blob column offsets (bf16 words per partition), in DMA/first-use order
OWK = 0
OXKV = OWK + 4096
OWV = OXKV + SK * 8
OWQ = OWV + 4096
OXQ = OWQ + 4096
OWO = OXQ + 16384
OMSK = OWO + 4096
BLOBW = OMSK + 16

_PROG = None


def _build_program():
    nc = bacc.Bacc("TRN2", target_bir_lowering=False)

    blob = nc.dram_tensor("blob", [128, BLOBW], bf16, kind="ExternalInput")
    y = nc.dram_tensor("y", [S, HID], fp32, kind="ExternalOutput")

    # SBUF arena (bf16 word offsets per partition), everything resident:
    #   KT   [0..4608)       K^T pair-major: KT[p, pair*1152 + sk]
    #   QT   [4608..12800)   Q^T: QT[p, pair*2048 + sq]
    #   YPN  [12800..20992)  normalized attn out^T: YPN[p, pair*2048 + sq]
    #   VP   [20992..30208)  pair*2304 + st*256 + [Va(64)|ma(64)|Vb(64)|mb(64)]
    #   WK   [30208..34304)  c-major weight chunks
    #   XKV  [34304..43520)  c-major: [:, c*1152 + sk]
    #   WV   [43520..47616)
    #   WQ   [47616..51712)
    #   XQ   [51712..68096)  q-major quarters, c-major within: q*4096 + c*512
    #   WO   [68096..72192)
    arena = nc.alloc_sbuf_tensor("arena", [128, 72192], bf16)
    base = nc.lookup_mloc(arena).addr

    def at(name, words, off_words):
        return nc.alloc_sbuf_tensor_at(
            name, [128, words], bf16, offset=base + off_words * 2
        )

    KT = at("KT", 4608, 0)
    QT = at("QT", 8192, 4608)
    YPN = at("YPN", 8192, 12800)
    VP = at("VP", 9216, 20992)
    WKs = at("WKs", 4096, 30208)
    XKVs = at("XKVs", 9216, 34304)
    WVs = at("WVs", 4096, 43520)
    WQs = at("WQs", 4096, 47616)
    XQs = at("XQs", 16384, 51712)
    WOs = at("WOs", 4096, 68096)

    with tile.TileContext(nc) as tc, ExitStack() as ctx:
        misc = ctx.enter_context(tc.tile_pool(name="misc", bufs=1))
        pt_pool = ctx.enter_context(tc.tile_pool(name="ptp", bufs=3))
        ev_pool = ctx.enter_context(tc.tile_pool(name="evp", bufs=3))
        rc_pool = ctx.enter_context(tc.tile_pool(name="rcp", bufs=2))
        ps_e = ctx.enter_context(tc.tile_pool(name="pse", bufs=3, space="PSUM"))
        ps_y = ctx.enter_context(tc.tile_pool(name="psy", bufs=2, space="PSUM"))

        masktb = misc.tile([128, 16], bf16)
        nc.sync.dma_start(masktb[:], blob[:, OMSK:OMSK + 16])
        maskt = misc.tile([128, 16], fp32)
        nc.vector.tensor_copy(maskt[:], masktb[:])

        # input stream from the blob, sliced by first use and split across
        # both HWDGE rings (sync + scalar) so transfers overlap compute
        nc.sync.dma_start(WKs[:], blob[:, OWK:OWK + 4096])
        for c in range(8):
            nc.sync.dma_start(
                XKVs[:, c * SK: c * SK + 384],
                blob[:, OXKV + c * SK: OXKV + c * SK + 384])
        for c in range(8):
            nc.scalar.dma_start(
                XKVs[:, c * SK + 384: c * SK + 768],
                blob[:, OXKV + c * SK + 384: OXKV + c * SK + 768])
        nc.sync.dma_start(WVs[:], blob[:, OWV:OWV + 4096])
        for c in range(8):
            nc.sync.dma_start(
                XKVs[:, c * SK + 768:(c + 1) * SK],
                blob[:, OXKV + c * SK + 768: OXKV + (c + 1) * SK])
        nc.scalar.dma_start(WQs[:], blob[:, OWQ:OWQ + 4096])
        nc.sync.dma_start(XQs[:, 0:4096], blob[:, OXQ:OXQ + 4096])
        nc.scalar.dma_start(XQs[:, 4096:8192],
                            blob[:, OXQ + 4096:OXQ + 8192])
        nc.sync.dma_start(XQs[:, 8192:12288],
                          blob[:, OXQ + 8192:OXQ + 12288])
        nc.scalar.dma_start(XQs[:, 12288:16384],
                            blob[:, OXQ + 12288:OXQ + 16384])
        nc.scalar.dma_start(WOs[:], blob[:, OWO:OWO + 4096])

        ones = misc.tile([128, 64], bf16)
        nc.vector.memset(ones[:], 1.0)

        # ---- Phase A: K^T -> KT, V (masked) -> VP ----
        def ktr_chunk(p, off, w):
            pk = ps_y.tile([128, 512], fp32, name="psyt")
            for c in range(8):
                nc.tensor.matmul(
                    pk[:, 0:w],
                    WKs[:, c * 512 + p * 128: c * 512 + (p + 1) * 128],
                    XKVs[:, c * SK + off: c * SK + off + w],
                    start=(c == 0), stop=(c == 7))
            nc.vector.tensor_copy(
                KT[:, p * SK + off: p * SK + off + w], pk[:, 0:w])

        def v_tile(st):
            pv = ps_e.tile([128, 1024], fp32, name="pe")
            for c in range(8):
                nc.tensor.matmul(
                    pv[:, 0:512],
                    XKVs[:, c * SK + st * 128: c * SK + (st + 1) * 128],
                    WVs[:, c * 512:(c + 1) * 512],
                    start=(c == 0), stop=(c == 7))
            for h in range(8):
                o = (h // 2) * 2304 + st * 256 + (h % 2) * 128
                nc.vector.tensor_scalar_mul(
                    VP[:, o:o + 64], pv[:, h * 64:(h + 1) * 64],
                    maskt[:, st:st + 1])

        for p in range(NPAIR):
            ktr_chunk(p, 0, 384)
        for p in range(NPAIR):
            ktr_chunk(p, 384, 384)
        for st in range(6):
            v_tile(st)
        for p in range(NPAIR):
            ktr_chunk(p, 768, 384)
        for st in range(6, SKT):
            v_tile(st)

        # ---- Phase B seed: Q^T for query block q0, all pairs ----
        def b_group(q, p):
            pq = ps_y.tile([128, 512], fp32, name="psyt")
            for c in range(8):
                nc.tensor.matmul(
                    pq[:],
                    WQs[:, c * 512 + p * 128: c * 512 + (p + 1) * 128],
                    XQs[:, q * 4096 + c * 512: q * 4096 + (c + 1) * 512],
                    start=(c == 0), stop=(c == 7))
            nc.vector.tensor_copy(
                QT[:, p * 2048 + q * 512: p * 2048 + (q + 1) * 512], pq[:])

        for p in range(NPAIR):
            b_group(0, p)

        # init VP mask columns (denominator ones, masked); stage-1 pairs
        # (2,3) first so the first combos' A*V reads are ready in time
        for p in (2, 3, 0, 1):
            for hh in range(2):
                for st in range(SKT):
                    o = p * 2304 + st * 256 + hh * 128 + 64
                    nc.vector.tensor_scalar_mul(
                        VP[:, o:o + 64], ones[:], maskt[:, st:st + 1])

        # ---- Phases C (attention) and D (out-proj), B fillers inside C ----
        def d_group(m, no):
            pd = ps_y.tile([128, 512], fp32, name="psyt")
            for tt in range(NPAIR):
                nc.tensor.matmul(
                    pd[:],
                    YPN[:, tt * 2048 + m * 128: tt * 2048 + (m + 1) * 128],
                    WOs[:, tt * 1024 + no * 512: tt * 1024 + no * 512 + 512],
                    start=(tt == 0), stop=(tt == 3))
            ob = ev_pool.tile([128, 512], fp32)
            nc.vector.tensor_copy(ob[:], pd[:])
            nc.sync.dma_start(
                y[m * 128:(m + 1) * 128, no * 512: no * 512 + 512], ob[:])

        NST = (2, 2, 2, 2, 1)  # sk tiles per exp stage (9 total)

        def c_combo(n, p, hh, host=None):
            # software-pipelined: pe/exp run 3 stages ahead of the A*V
            # matmuls; hosted filler/out-proj work runs in the exp ramp so
            # PE never waits on ACT.  py is allocated AFTER host() so the
            # 2-buf psum rotation never recycles an open accumulator.
            qsl = QT[hh * 64:(hh + 1) * 64,
                     p * 2048 + n * 512: p * 2048 + n * 512 + 512]
            pts = []

            def emit_pe(k):
                pe = ps_e.tile([128, 1024], fp32)
                w = 512 * NST[k]
                for j in range(NST[k]):
                    st = 2 * k + j
                    nc.tensor.matmul(
                        pe[:, j * 512:(j + 1) * 512],
                        KT[hh * 64:(hh + 1) * 64,
                           p * SK + st * 128: p * SK + (st + 1) * 128],
                        qsl, start=True, stop=True,
                        tile_position=(hh * 64, 0))
                pt = pt_pool.tile([128, 1024], bf16)
                nc.scalar.activation(pt[:, 0:w], pe[:, 0:w], Exp, scale=0.125)
                pts.append(pt)

            def emit_py(k, py):
                for j in range(NST[k]):
                    st = 2 * k + j
                    nc.tensor.matmul(
                        py[:],
                        VP[:, p * 2304 + st * 256 + hh * 128:
                           p * 2304 + st * 256 + hh * 128 + 128],
                        pts[k][:, j * 512:(j + 1) * 512],
                        start=(st == 0), stop=(st == SKT - 1))

            for k in range(3):
                emit_pe(k)
            if host is not None:
                host()
            py = ps_y.tile([128, 512], fp32, name="psyt")
            for k in range(3, 5):
                emit_py(k - 3, py)
                emit_pe(k)
            for k in range(2, 5):
                emit_py(k, py)
            rc = rc_pool.tile([64, 512], fp32)
            nc.vector.reciprocal(rc[:], py[64:128, :])
            nc.vector.tensor_mul(
                YPN[hh * 64:(hh + 1) * 64,
                    p * 2048 + n * 512: p * 2048 + n * 512 + 512],
                py[0:64, :], rc[:])

        # B fillers for stage 1: one group behind each combo; each (q,p)
        # filler precedes the first stage-1 combo reading QT(q,p).
        fillers = [(1, 2), (1, 3), (2, 2), (2, 3), (1, 0), (1, 1),
                   (2, 0), (3, 2), (3, 3), (2, 1), (3, 0), (3, 1)]

        def filler_host(i):
            def host():
                fq, fp = fillers[i]
                b_group(fq, fp)
            return host

        def d_host(groups):
            def host():
                for m, no in groups:
                    d_group(m, no)
            return host

        with nc.allow_low_precision(reason="bf16 within tolerance"):
            # Stage 1: head pairs 2-3 over all query blocks, B fillers inside
            idx = 0
            for n in range(4):
                for p in (2, 3):
                    for hh in range(2):
                        c_combo(n, p, hh,
                                filler_host(idx) if idx < len(fillers)
                                else None)
                        idx += 1

            # Stage 2: head pairs 0-1; D(n-1) groups ride in block n's shadow
            for n in range(4):
                dlist = ([(m, no) for m in range((n - 1) * 4, n * 4)
                          for no in range(2)] if n >= 1 else [])
                di = 0
                for p in (0, 1):
                    for hh in range(2):
                        c_combo(n, p, hh,
                                d_host(dlist[di:di + 2]) if di < len(dlist)
                                else None)
                        di += 2
            for m in range(12, 16):
                for no in range(2):
                    d_group(m, no)

    nc.finalize()
    return nc


def _get_program():
    global _PROG
    if _PROG is None:
        _PROG = _build_program()
    return _PROG


def _make_in_maps(inputs):
    from ml_dtypes import bfloat16
    X_Q = np.asarray(inputs["X_Q"], dtype=np.float32)
    X_KV = np.asarray(inputs["X_KV"], dtype=np.float32)
    mask = np.asarray(inputs["key_padding_mask"])
    W_Q = np.asarray(inputs["W_Q"], dtype=np.float32)
    W_K = np.asarray(inputs["W_K"], dtype=np.float32)
    W_V = np.asarray(inputs["W_V"], dtype=np.float32)
    W_O = np.asarray(inputs["W_O"], dtype=np.float32)
    in_maps = []
    for core in range(8):
        b, half = core // 2, core % 2
        idx = np.flatnonzero(~mask[b].astype(bool))
        nk = len(idx)
        assert nk <= SK, f"kept keys {nk} exceed padded SK={SK}"
        xkvc = np.zeros((SK, HID), dtype=np.float32)
        xkvc[:nk] = X_KV[b][idx]
        maskv = (np.arange(SK) < nk).astype(np.float32)

        def wimg(W):
            return (W[:, half * 512:(half + 1) * 512]
                    .reshape(8, 128, 512).transpose(1, 0, 2).reshape(128, 4096))

        blob = np.zeros((128, BLOBW), dtype=bfloat16)
        blob[:, OWK:OWK + 4096] = wimg(W_K)
        blob[:, OXKV:OXKV + SK * 8] = \
            xkvc.reshape(SK, 8, 128).transpose(2, 1, 0).reshape(128, SK * 8)
        blob[:, OWV:OWV + 4096] = wimg(W_V)
        blob[:, OWQ:OWQ + 4096] = wimg(W_Q)
        blob[:, OXQ:OXQ + 16384] = \
            X_Q[b].reshape(4, 512, 8, 128).transpose(3, 0, 2, 1).reshape(128, 16384)
        blob[:, OWO:OWO + 4096] = \
            (W_O[half * 512:(half + 1) * 512]
             .reshape(4, 128, 1024).transpose(1, 0, 2).reshape(128, 4096))
        blob[:, OMSK:OMSK + SKT] = maskv.reshape(SKT, 128).T
        in_maps.append({"blob": blob})
    return in_maps


def kernel(**inputs):
    nc = _get_program()
    in_maps = _make_in_maps(inputs)
    res = run_bass_kernel_spmd(nc, in_maps, core_ids=list(range(8)))
    out = np.empty((B, S, HID), dtype=np.float32)
    for b in range(B):
        out[b] = res.results[2 * b]["y"] + res.results[2 * b + 1]["y"]
    return out



# revision 26
# speedup vs baseline: 299.8913x; 299.8913x over previous
import sys

sys.path.insert(0, "/opt/trn_rl_repo")
import numpy as np
from contextlib import ExitStack

from concourse import bacc
import concourse.tile as tile
from concourse import mybir
from concourse.bass_utils import run_bass_kernel_spmd

fp32 = mybir.dt.float32
bf16 = mybir.dt.bfloat16
Exp = mybir.ActivationFunctionType.Exp

B, S, HID = 4, 2048, 1024
H, DK = 16, 64
SK = 1152          # compacted+padded key count (max kept keys = 1036)
SKT = SK // 128    # 9 sk tiles
NPAIR = 4          # head pairs per core (8 heads = half the 16)

# blob column offsets (bf16 words per partition), in DMA/first-use order
OWK = 0
OXKV = OWK + 4096
OWV = OXKV + SK * 8
OWQ = OWV + 4096
OXQ = OWQ + 4096
OWO = OXQ + 16384
OMSK = OWO + 4096
OVPM = OMSK + 16    # masked-ones image for VP denominator columns
BLOBW = OVPM + 576

_PROG = None


def _build_program():
    nc = bacc.Bacc("TRN2", target_bir_lowering=False)

    blob = nc.dram_tensor("blob", [128, BLOBW], bf16, kind="ExternalInput")
    y = nc.dram_tensor("y", [S, HID], fp32, kind="ExternalOutput")

    # SBUF arena (bf16 word offsets per partition), everything resident:
    #   KT   [0..4608)       K^T pair-major: KT[p, pair*1152 + sk]
    #   QT   [4608..12800)   Q^T: QT[p, pair*2048 + sq]
    #   YPN  [12800..20992)  normalized attn out^T: YPN[p, pair*2048 + sq]
    #   VP   [20992..30208)  pair*2304 + st*256 + [Va(64)|ma(64)|Vb(64)|mb(64)]
    #   WK   [30208..34304)  c-major weight chunks
    #   XKV  [34304..43520)  c-major: [:, c*1152 + sk]
    #   WV   [43520..47616)
    #   WQ   [47616..51712)
    #   XQ   [51712..68096)  q-major quarters, c-major within: q*4096 + c*512
    #   WO   [68096..72192)
    arena = nc.alloc_sbuf_tensor("arena", [128, 72192], bf16)
    base = nc.lookup_mloc(arena).addr

    def at(name, words, off_words):
        return nc.alloc_sbuf_tensor_at(
            name, [128, words], bf16, offset=base + off_words * 2
        )

    KT = at("KT", 4608, 0)
    QT = at("QT", 8192, 4608)
    YPN = at("YPN", 8192, 12800)
    VP = at("VP", 9216, 20992)
    WKs = at("WKs", 4096, 30208)
    XKVs = at("XKVs", 9216, 34304)
    WVs = at("WVs", 4096, 43520)
    WQs = at("WQs", 4096, 47616)
    XQs = at("XQs", 16384, 51712)
    WOs = at("WOs", 4096, 68096)

    with tile.TileContext(nc) as tc, ExitStack() as ctx:
        misc = ctx.enter_context(tc.tile_pool(name="misc", bufs=1))
        # pe-part(i+1) fills its 9 exp tiles while py-part(i) still holds
        # its 9, so the exp-tile pool must cover two full combos.
        pt_pool = ctx.enter_context(tc.tile_pool(name="ptp", bufs=18))
        ev_pool = ctx.enter_context(tc.tile_pool(name="evp", bufs=2))
        rc_pool = ctx.enter_context(tc.tile_pool(name="rcp", bufs=1))
        # PSUM bank budget (8 banks total):
        #   ps_e  2 bufs x [128,1024] fp32 = 4 banks   (E tiles + A-phase pv)
        #   ps_py 1 buf  x [128,1024] fp32 = 2 banks   (A*V accumulators, both heads)
        #   ps_h  2 bufs x [128, 512] fp32 = 2 banks   (proj groups: ktr/B/D hosts)
        ps_e = ctx.enter_context(tc.tile_pool(name="pse", bufs=2, space="PSUM"))
        ps_py = ctx.enter_context(tc.tile_pool(name="pspy", bufs=1, space="PSUM"))
        ps_h = ctx.enter_context(tc.tile_pool(name="psh", bufs=2, space="PSUM"))

        masktb = misc.tile([128, 16], bf16)
        nc.sync.dma_start(masktb[:], blob[:, OMSK:OMSK + 16])
        maskt = misc.tile([128, 16], fp32)
        nc.vector.tensor_copy(maskt[:], masktb[:])

        # input stream from the blob, sliced by first use and split across
        # both HWDGE rings (sync + scalar). Weight images are PAIR-major
        # (pair*1024 + c*128), so only the pair-2 columns of W_K/W_Q ride
        # the critical path; X_KV moves as per-c chunks (2.3KB/partition
        # lines, 4 per ring) so the whole K/V input lands ~15us in.
        nc.sync.dma_start(WKs[:, 2048:3072], blob[:, OWK + 2048:OWK + 3072])
        nc.scalar.dma_start(WQs[:, 2048:3072],
                            blob[:, OWQ + 2048:OWQ + 3072])
        nc.scalar.dma_start(XQs[:, 0:4096], blob[:, OXQ:OXQ + 4096])
        for c in range(4):
            nc.sync.dma_start(
                XKVs[:, c * SK:(c + 1) * SK],
                blob[:, OXKV + c * SK: OXKV + (c + 1) * SK])
        for c in range(4, 8):
            nc.scalar.dma_start(
                XKVs[:, c * SK:(c + 1) * SK],
                blob[:, OXKV + c * SK: OXKV + (c + 1) * SK])
        nc.sync.dma_start(WKs[:, 0:2048], blob[:, OWK:OWK + 2048])
        nc.sync.dma_start(WKs[:, 3072:4096], blob[:, OWK + 3072:OWK + 4096])
        nc.sync.dma_start(WQs[:, 0:2048], blob[:, OWQ:OWQ + 2048])
        nc.sync.dma_start(WQs[:, 3072:4096], blob[:, OWQ + 3072:OWQ + 4096])
        nc.scalar.dma_start(WVs[:], blob[:, OWV:OWV + 4096])
        nc.scalar.dma_start(XQs[:, 4096:8192],
                            blob[:, OXQ + 4096:OXQ + 8192])
        nc.scalar.dma_start(XQs[:, 8192:12288],
                            blob[:, OXQ + 8192:OXQ + 12288])
        nc.scalar.dma_start(XQs[:, 12288:16384],
                            blob[:, OXQ + 12288:OXQ + 16384])
        nc.scalar.dma_start(WOs[:], blob[:, OWO:OWO + 4096])

        # ---- Phase A: K^T -> KT, V (masked) -> VP ----
        def ktr_chunk(p, off, w):
            pk = ps_h.tile([128, 512], fp32, name="psh")
            for c in range(8):
                nc.tensor.matmul(
                    pk[:, 0:w],
                    WKs[:, p * 1024 + c * 128: p * 1024 + (c + 1) * 128],
                    XKVs[:, c * SK + off: c * SK + off + w],
                    start=(c == 0), stop=(c == 7))
            nc.vector.tensor_copy(
                KT[:, p * SK + off: p * SK + off + w], pk[:, 0:w])

        def v_tile(st):
            pv = ps_e.tile([128, 1024], fp32, name="pe")
            for c in range(8):
                nc.tensor.matmul(
                    pv[:, 0:512],
                    XKVs[:, c * SK + st * 128: c * SK + (st + 1) * 128],
                    WVs[:, c * 512:(c + 1) * 512],
                    start=(c == 0), stop=(c == 7))
            for h in range(8):
                o = (h // 2) * 2304 + st * 256 + (h % 2) * 128 + 64
                nc.vector.tensor_scalar_mul(
                    VP[:, o:o + 64], pv[:, h * 64:(h + 1) * 64],
                    maskt[:, st:st + 1])

        # ---- Q^T projection group ----
        def b_group(q, p):
            pq = ps_h.tile([128, 512], fp32, name="psh")
            for c in range(8):
                nc.tensor.matmul(
                    pq[:],
                    WQs[:, p * 1024 + c * 128: p * 1024 + (c + 1) * 128],
                    XQs[:, q * 4096 + c * 512: q * 4096 + (c + 1) * 512],
                    start=(c == 0), stop=(c == 7))
            nc.vector.tensor_copy(
                QT[:, p * 2048 + q * 512: p * 2048 + (q + 1) * 512], pq[:])

        # init VP mask columns (denominator ones, masked, in the LOW 64 cols
        # of each 128 block so the denominator lands in psum partitions 0:64
        # where reciprocal_approx_fast works) straight from a host-built
        # image — keeps the DVE free for the critical K^T/Q^T copies.
        vpm_src = blob[:, OVPM:OVPM + 576].rearrange(
            "a (s c) -> a s c", s=SKT)
        for i, p in enumerate((2, 3, 0, 1)):
            for hh in range(2):
                dst = VP[:, p * 2304:(p + 1) * 2304].rearrange(
                    "a (s c) -> a s c", s=SKT)[:, :, hh * 128: hh * 128 + 64]
                eng = nc.sync if i % 2 == 0 else nc.scalar
                eng.dma_start(dst, vpm_src)

        # ---- Phases C (attention) and D (out-proj), B/D hosted inside C ----
        def d_group(m, no):
            pd = ps_h.tile([128, 512], fp32, name="psh")
            for tt in range(NPAIR):
                nc.tensor.matmul(
                    pd[:],
                    YPN[:, tt * 2048 + m * 128: tt * 2048 + (m + 1) * 128],
                    WOs[:, tt * 1024 + no * 512: tt * 1024 + no * 512 + 512],
                    start=(tt == 0), stop=(tt == 3))
            ob = ev_pool.tile([128, 512], fp32)
            nc.vector.tensor_copy(ob[:], pd[:])
            nc.sync.dma_start(
                y[m * 128:(m + 1) * 128, no * 512: no * 512 + 512], ob[:])

        def c_pe_part(n, p, hosts=None):
            # E + exp pass for both heads (hh=0,1) of pair p, query block n.
            # The two E matmuls of each key tile go to row groups 0 and 64
            # of the PE array (tile_position) and run CONCURRENTLY; exp runs
            # one key tile behind, pacing the whole kernel. PE is ~80% idle
            # here, so `hosts` (projection filler groups, keyed by stage)
            # ride in the slack. Returns the exp tiles for the py pass.
            qsl = [QT[hh * 64: (hh + 1) * 64,
                      p * 2048 + n * 512: p * 2048 + n * 512 + 512]
                   for hh in (0, 1)]
            pts = []
            for st in range(SKT):
                pe = ps_e.tile([128, 1024], fp32, name="pe")
                for hh in (0, 1):
                    nc.tensor.matmul(
                        pe[:, hh * 512:(hh + 1) * 512],
                        KT[hh * 64:(hh + 1) * 64,
                           p * SK + st * 128: p * SK + (st + 1) * 128],
                        qsl[hh], start=True, stop=True,
                        tile_position=(hh * 64, 0))
                pt = pt_pool.tile([128, 1024], bf16)
                nc.scalar.activation(pt[:], pe[:], Exp, scale=0.125)
                pts.append(pt)
                if hosts and st in hosts:
                    for fn in hosts[st]:
                        fn()
            return pts

        def c_py_part(n, p, pts):
            # A*V pass: accumulate both heads' (A@V | denominator) over all
            # key tiles, then normalize into YPN. Emitted one combo behind
            # its pe-part so its matmuls fill PE slack of the next pe-part.
            py = ps_py.tile([128, 1024], fp32, name="pspy")
            for st in range(SKT):
                for hh in (0, 1):
                    nc.tensor.matmul(
                        py[:, hh * 512:(hh + 1) * 512],
                        VP[:, p * 2304 + st * 256 + hh * 128:
                           p * 2304 + st * 256 + hh * 128 + 128],
                        pts[st][:, hh * 512:(hh + 1) * 512],
                        start=(st == 0), stop=(st == SKT - 1))
            rc = rc_pool.tile([64, 1024], fp32)
            nc.vector.reciprocal_approx_fast(rc[:], py[0:64, :])
            for hh in (0, 1):
                nc.vector.tensor_mul(
                    YPN[hh * 64:(hh + 1) * 64,
                        p * 2048 + n * 512: p * 2048 + n * 512 + 512],
                    py[64:128, hh * 512:(hh + 1) * 512],
                    rc[:, hh * 512:(hh + 1) * 512])

        def K_(p, off):
            return lambda: ktr_chunk(p, off, 384)

        def B_(q, p):
            return lambda: b_group(q, p)

        def V_(st):
            return lambda: v_tile(st)

        def D_(m, no):
            return lambda: d_group(m, no)

        with nc.allow_low_precision(reason="bf16 within tolerance"):
            # minimal head: K^T(pair2, first key third) + Q^T(block0,
            # pair2), then the exp pipeline starts; everything else —
            # including the rest of K^T(pair2) — is hosted inside pe-parts.
            ktr_chunk(2, 0, 384)
            b_group(0, 2)

            # Global software pipeline: pe-part(i+1) is emitted before
            # py-part(i), so the Scalar engine always has a full combo of
            # exp work queued while PE retires the previous combo's A*V
            # matmuls plus hosted projection groups.
            # NOTE: Tile derives dependencies from EMISSION order, so every
            # producer (V/K/B/D input) must be emitted before its first
            # reader: V tiles inside pe0/pe1 (before py0); K^T(p) and
            # Q^T(n,p) before pe-part(n,p); D(block b) after py-part(b, p1).
            sched = [
                ((0, 2), {0: [K_(2, 384)], 1: [K_(2, 768)], 2: [V_(0)],
                          3: [V_(1)], 4: [V_(2)], 5: [V_(3)], 6: [V_(4)],
                          7: [B_(1, 2)]}),
                ((1, 2), {0: [V_(5)], 1: [V_(6)], 2: [V_(7)], 3: [V_(8)],
                          5: [B_(2, 2)]}),
                ((2, 2), {0: [B_(3, 2)], 2: [K_(3, 0)]}),
                ((3, 2), {0: [K_(3, 384)], 2: [K_(3, 768)],
                          4: [B_(0, 3)]}),
                ((0, 3), {0: [B_(1, 3)], 2: [K_(0, 0)]}),
                ((1, 3), {0: [B_(2, 3)], 2: [K_(0, 384)]}),
                ((2, 3), {0: [B_(3, 3)], 2: [K_(0, 768)],
                          4: [K_(1, 0)]}),
                ((3, 3), {0: [K_(1, 384)], 2: [K_(1, 768)],
                          4: [B_(0, 0)]}),
                ((0, 0), {0: [B_(0, 1)], 2: [B_(1, 0)]}),
                ((0, 1), {0: [B_(1, 1)]}),
                ((1, 0), {0: [B_(2, 0)]}),
                ((1, 1), {0: [B_(2, 1)], 2: [D_(0, 0)], 4: [D_(0, 1)],
                          6: [D_(1, 0)], 8: [D_(1, 1)]}),
                ((2, 0), {0: [B_(3, 0)], 2: [D_(2, 0)], 4: [D_(2, 1)],
                          6: [D_(3, 0)], 8: [D_(3, 1)]}),
                ((2, 1), {0: [B_(3, 1)], 2: [D_(4, 0)], 4: [D_(4, 1)],
                          6: [D_(5, 0)], 8: [D_(5, 1)]}),
                ((3, 0), {0: [D_(6, 0)], 2: [D_(6, 1)], 4: [D_(7, 0)],
                          6: [D_(7, 1)]}),
                ((3, 1), {0: [D_(8, 0)], 1: [D_(8, 1)], 2: [D_(9, 0)],
                          3: [D_(9, 1)], 4: [D_(10, 0)], 5: [D_(10, 1)],
                          6: [D_(11, 0)], 7: [D_(11, 1)]}),
            ]
            pending = None
            for (n, p), hosts in sched:
                pts = c_pe_part(n, p, hosts)
                if pending is not None:
                    c_py_part(*pending)
                pending = (n, p, pts)
            c_py_part(*pending)
            for m in range(12, 16):
                for no in range(2):
                    d_group(m, no)

    nc.finalize()
    return nc


def _get_program():
    global _PROG
    if _PROG is None:
        _PROG = _build_program()
    return _PROG


def _make_in_maps(inputs):
    from ml_dtypes import bfloat16
    X_Q = np.asarray(inputs["X_Q"], dtype=np.float32)
    X_KV = np.asarray(inputs["X_KV"], dtype=np.float32)
    mask = np.asarray(inputs["key_padding_mask"])
    W_Q = np.asarray(inputs["W_Q"], dtype=np.float32)
    W_K = np.asarray(inputs["W_K"], dtype=np.float32)
    W_V = np.asarray(inputs["W_V"], dtype=np.float32)
    W_O = np.asarray(inputs["W_O"], dtype=np.float32)
    in_maps = []
    for core in range(8):
        b, half = core // 2, core % 2
        idx = np.flatnonzero(~mask[b].astype(bool))
        nk = len(idx)
        assert nk <= SK, f"kept keys {nk} exceed padded SK={SK}"
        xkvc = np.zeros((SK, HID), dtype=np.float32)
        xkvc[:nk] = X_KV[b][idx]
        maskv = (np.arange(SK) < nk).astype(np.float32)

        def wimg(W):
            # c-major image (moving operand): col = c*512 + m
            return (W[:, half * 512:(half + 1) * 512]
                    .reshape(8, 128, 512).transpose(1, 0, 2).reshape(128, 4096))

        def wimg_pair(W):
            # pair-major image (stationary per (pair, c)): col =
            # pair*1024 + c*128 + j, value W[c*128 + p, half*512 + pair*128 + j]
            return (W[:, half * 512:(half + 1) * 512]
                    .reshape(8, 128, 4, 128).transpose(1, 2, 0, 3)
                    .reshape(128, 4096))

        blob = np.zeros((128, BLOBW), dtype=bfloat16)
        blob[:, OWK:OWK + 4096] = wimg_pair(W_K)
        blob[:, OXKV:OXKV + SK * 8] = \
            xkvc.reshape(SK, 8, 128).transpose(2, 1, 0).reshape(128, SK * 8)
        blob[:, OWV:OWV + 4096] = wimg(W_V)
        blob[:, OWQ:OWQ + 4096] = wimg_pair(W_Q)
        blob[:, OXQ:OXQ + 16384] = \
            X_Q[b].reshape(4, 512, 8, 128).transpose(3, 0, 2, 1).reshape(128, 16384)
        blob[:, OWO:OWO + 4096] = \
            (W_O[half * 512:(half + 1) * 512]
             .reshape(4, 128, 1024).transpose(1, 0, 2).reshape(128, 4096))
        blob[:, OMSK:OMSK + SKT] = maskv.reshape(SKT, 128).T
        blob[:, OVPM:OVPM + 576] = np.repeat(
            maskv.reshape(SKT, 128).T[:, :, None], 64, axis=2
        ).reshape(128, SKT * 64)
        in_maps.append({"blob": blob})
    return in_maps


def kernel(**inputs):
    nc = _get_program()
    in_maps = _make_in_maps(inputs)
    res = run_bass_kernel_spmd(nc, in_maps, core_ids=list(range(8)))
    out = np.empty((B, S, HID), dtype=np.float32)
    for b in range(B):
        out[b] = res.results[2 * b]["y"] + res.results[2 * b + 1]["y"]
    return out


# revision 33
# speedup vs baseline: 308.1523x; 1.0275x over previous
import sys

sys.path.insert(0, "/opt/trn_rl_repo")
import numpy as np
from contextlib import ExitStack

from concourse import bacc
import concourse.tile as tile
from concourse import mybir
from concourse.bass_utils import run_bass_kernel_spmd

fp32 = mybir.dt.float32
bf16 = mybir.dt.bfloat16
Exp = mybir.ActivationFunctionType.Exp

B, S, HID = 4, 2048, 1024
H, DK = 16, 64
SK = 1152          # compacted+padded key count (max kept keys = 1036)
SKT = SK // 128    # 9 sk tiles
NPAIR = 4          # head pairs per core (8 heads = half the 16)

# blob column offsets (bf16 words per partition), in DMA/first-use order
OWK = 0
OXKV = OWK + 4096
OWV = OXKV + SK * 8
OWQ = OWV + 4096
OXQ = OWQ + 4096
OWO = OXQ + 16384
OMSK = OWO + 4096
OVPM = OMSK + 16    # masked-ones image for VP denominator columns
BLOBW = OVPM + 576

_PROG = None


def _build_program():
    nc = bacc.Bacc("TRN2", target_bir_lowering=False)

    blob = nc.dram_tensor("blob", [128, BLOBW], bf16, kind="ExternalInput")
    y = nc.dram_tensor("y", [S, HID], fp32, kind="ExternalOutput")

    # SBUF arena (bf16 word offsets per partition), everything resident:
    #   KT   [0..4608)       K^T pair-major: KT[p, pair*1152 + sk]
    #   QT   [4608..12800)   Q^T: QT[p, pair*2048 + sq]
    #   YPN  [12800..20992)  normalized attn out^T: YPN[p, pair*2048 + sq]
    #   VP   [20992..30208)  pair*2304 + st*256 + [Va(64)|ma(64)|Vb(64)|mb(64)]
    #   WK   [30208..34304)  c-major weight chunks
    #   XKV  [34304..43520)  c-major: [:, c*1152 + sk]
    #   WV   [43520..47616)
    #   WQ   [47616..51712)
    #   XQ   [51712..68096)  q-major quarters, c-major within: q*4096 + c*512
    #   WO   [68096..72192)
    arena = nc.alloc_sbuf_tensor("arena", [128, 72192], bf16)
    base = nc.lookup_mloc(arena).addr

    def at(name, words, off_words):
        return nc.alloc_sbuf_tensor_at(
            name, [128, words], bf16, offset=base + off_words * 2
        )

    KT = at("KT", 4608, 0)
    QT = at("QT", 8192, 4608)
    YPN = at("YPN", 8192, 12800)
    VP = at("VP", 9216, 20992)
    WKs = at("WKs", 4096, 30208)
    XKVs = at("XKVs", 9216, 34304)
    WVs = at("WVs", 4096, 43520)
    WQs = at("WQs", 4096, 47616)
    XQs = at("XQs", 16384, 51712)
    WOs = at("WOs", 4096, 68096)

    with tile.TileContext(nc) as tc, ExitStack() as ctx:
        misc = ctx.enter_context(tc.tile_pool(name="misc", bufs=1))
        # pe-part(i+1) fills its 9 exp tiles while py-part(i) still holds
        # its 9, so the exp-tile pool must cover two full combos.
        pt_pool = ctx.enter_context(tc.tile_pool(name="ptp", bufs=18))
        ev_pool = ctx.enter_context(tc.tile_pool(name="evp", bufs=2))
        rc_pool = ctx.enter_context(tc.tile_pool(name="rcp", bufs=1))
        # PSUM bank budget (8 banks total):
        #   ps_e  2 bufs x [128,1024] fp32 = 4 banks   (E tiles + A-phase pv)
        #   ps_py 1 buf  x [128,1024] fp32 = 2 banks   (A*V accumulators, both heads)
        #   ps_h  2 bufs x [128, 512] fp32 = 2 banks   (proj groups: ktr/B/D hosts)
        ps_e = ctx.enter_context(tc.tile_pool(name="pse", bufs=2, space="PSUM"))
        ps_py = ctx.enter_context(tc.tile_pool(name="pspy", bufs=1, space="PSUM"))
        ps_h = ctx.enter_context(tc.tile_pool(name="psh", bufs=2, space="PSUM"))

        masktb = misc.tile([128, 16], bf16)
        nc.sync.dma_start(masktb[:], blob[:, OMSK:OMSK + 16])
        maskt = misc.tile([128, 16], fp32)
        nc.vector.tensor_copy(maskt[:], masktb[:])

        # input stream from the blob, sliced by first use and split across
        # the sync + GPSIMD HWDGE rings. The Scalar engine issues NO DMA
        # descriptors: every dma_start occupies ~0.6us of its in-order
        # queue, which would push the first exp out by ~20us. Weight images
        # are PAIR-major (pair*1024 + c*128), so only the pair-2 columns of
        # W_K/W_Q ride the critical path; X_KV moves as per-c chunks
        # (2.3KB/partition lines, 4 per ring).
        nc.sync.dma_start(WKs[:, 2048:3072], blob[:, OWK + 2048:OWK + 3072])
        nc.gpsimd.dma_start(WQs[:, 2048:3072],
                            blob[:, OWQ + 2048:OWQ + 3072])
        nc.gpsimd.dma_start(XQs[:, 0:4096], blob[:, OXQ:OXQ + 4096])
        for c in range(4):
            nc.sync.dma_start(
                XKVs[:, c * SK:(c + 1) * SK],
                blob[:, OXKV + c * SK: OXKV + (c + 1) * SK])
        for c in range(4, 8):
            nc.gpsimd.dma_start(
                XKVs[:, c * SK:(c + 1) * SK],
                blob[:, OXKV + c * SK: OXKV + (c + 1) * SK])
        nc.gpsimd.dma_start(WVs[:], blob[:, OWV:OWV + 4096])
        nc.gpsimd.dma_start(XQs[:, 4096:8192],
                            blob[:, OXQ + 4096:OXQ + 8192])
        nc.sync.dma_start(WKs[:, 0:2048], blob[:, OWK:OWK + 2048])
        nc.sync.dma_start(WKs[:, 3072:4096], blob[:, OWK + 3072:OWK + 4096])
        nc.sync.dma_start(WQs[:, 0:2048], blob[:, OWQ:OWQ + 2048])
        nc.sync.dma_start(WQs[:, 3072:4096], blob[:, OWQ + 3072:OWQ + 4096])
        nc.sync.dma_start(XQs[:, 8192:12288],
                          blob[:, OXQ + 8192:OXQ + 12288])
        nc.gpsimd.dma_start(XQs[:, 12288:16384],
                            blob[:, OXQ + 12288:OXQ + 16384])
        nc.gpsimd.dma_start(WOs[:], blob[:, OWO:OWO + 4096])

        ones = misc.tile([128, 64], bf16)
        nc.vector.memset(ones[:], 1.0)

        # ---- Phase A: K^T -> KT, V (masked) -> VP ----
        def ktr_chunk(p, off, w):
            pk = ps_h.tile([128, 512], fp32, name="psh")
            for c in range(8):
                nc.tensor.matmul(
                    pk[:, 0:w],
                    WKs[:, p * 1024 + c * 128: p * 1024 + (c + 1) * 128],
                    XKVs[:, c * SK + off: c * SK + off + w],
                    start=(c == 0), stop=(c == 7))
            nc.vector.tensor_copy(
                KT[:, p * SK + off: p * SK + off + w], pk[:, 0:w])

        def v_tile(st):
            pv = ps_e.tile([128, 1024], fp32, name="pe")
            for c in range(8):
                nc.tensor.matmul(
                    pv[:, 0:512],
                    XKVs[:, c * SK + st * 128: c * SK + (st + 1) * 128],
                    WVs[:, c * 512:(c + 1) * 512],
                    start=(c == 0), stop=(c == 7))
            for h in range(8):
                o = (h // 2) * 2304 + st * 256 + (h % 2) * 128 + 64
                nc.vector.tensor_scalar_mul(
                    VP[:, o:o + 64], pv[:, h * 64:(h + 1) * 64],
                    maskt[:, st:st + 1])

        # ---- Q^T projection group ----
        def b_group(q, p):
            pq = ps_h.tile([128, 512], fp32, name="psh")
            for c in range(8):
                nc.tensor.matmul(
                    pq[:],
                    WQs[:, p * 1024 + c * 128: p * 1024 + (c + 1) * 128],
                    XQs[:, q * 4096 + c * 512: q * 4096 + (c + 1) * 512],
                    start=(c == 0), stop=(c == 7))
            nc.vector.tensor_copy(
                QT[:, p * 2048 + q * 512: p * 2048 + (q + 1) * 512], pq[:])

        # VP mask columns (denominator ones, masked, in the LOW 64 cols of
        # each 128 block so the denominator lands in psum partitions 0:64
        # where reciprocal_approx_fast works) are initialized per pair by
        # O_() hosts inside early pe-parts — after the critical K^T/Q^T
        # copies in the DVE queue, before the pair's first py-part.

        # ---- Phases C (attention) and D (out-proj), B/D hosted inside C ----
        def d_group(m, no):
            pd = ps_h.tile([128, 512], fp32, name="psh")
            for tt in range(NPAIR):
                nc.tensor.matmul(
                    pd[:],
                    YPN[:, tt * 2048 + m * 128: tt * 2048 + (m + 1) * 128],
                    WOs[:, tt * 1024 + no * 512: tt * 1024 + no * 512 + 512],
                    start=(tt == 0), stop=(tt == 3))
            ob = ev_pool.tile([128, 512], fp32)
            nc.vector.tensor_copy(ob[:], pd[:])
            nc.sync.dma_start(
                y[m * 128:(m + 1) * 128, no * 512: no * 512 + 512], ob[:])

        def py_finish(n, p, py):
            rc = rc_pool.tile([64, 1024], fp32)
            nc.vector.reciprocal_approx_fast(rc[:], py[0:64, :])
            for hh in (0, 1):
                nc.vector.tensor_mul(
                    YPN[hh * 64:(hh + 1) * 64,
                        p * 2048 + n * 512: p * 2048 + n * 512 + 512],
                    py[64:128, hh * 512:(hh + 1) * 512],
                    rc[:, hh * 512:(hh + 1) * 512])

        def py_stage(p, py, pts, st):
            for hh in (0, 1):
                nc.tensor.matmul(
                    py[:, hh * 512:(hh + 1) * 512],
                    VP[:, p * 2304 + st * 256 + hh * 128:
                       p * 2304 + st * 256 + hh * 128 + 128],
                    pts[st][:, hh * 512:(hh + 1) * 512],
                    start=(st == 0), stop=(st == SKT - 1))

        def c_part(n, p, hosts=None, prev=None):
            # E + exp pass for both heads (hh=0,1) of pair p, query block n.
            # The two E matmuls of each key tile go to row groups 0 and 64
            # of the PE array (tile_position) and run CONCURRENTLY; exp runs
            # one key tile behind, pacing the whole kernel. The PREVIOUS
            # combo's A*V matmuls are interleaved stage-by-stage (a block of
            # 18 of them between combos starves the exp queue), and `hosts`
            # (projection filler groups, keyed by stage) ride in the
            # remaining PE slack. Returns the exp tiles for the next pass.
            qsl = [QT[hh * 64: (hh + 1) * 64,
                      p * 2048 + n * 512: p * 2048 + n * 512 + 512]
                   for hh in (0, 1)]
            pts = []
            ppy = None
            if prev is not None:
                pn, pp, ppts = prev
                ppy = ps_py.tile([128, 1024], fp32, name="pspy")
            for st in range(SKT):
                pe = ps_e.tile([128, 1024], fp32, name="pe")
                for hh in (0, 1):
                    nc.tensor.matmul(
                        pe[:, hh * 512:(hh + 1) * 512],
                        KT[hh * 64:(hh + 1) * 64,
                           p * SK + st * 128: p * SK + (st + 1) * 128],
                        qsl[hh], start=True, stop=True,
                        tile_position=(hh * 64, 0))
                pt = pt_pool.tile([128, 1024], bf16)
                nc.scalar.activation(pt[:], pe[:], Exp, scale=0.125)
                pts.append(pt)
                if prev is not None:
                    py_stage(pp, ppy, ppts, st)
                if hosts and st in hosts:
                    for fn in hosts[st]:
                        fn()
            if prev is not None:
                py_finish(pn, pp, ppy)
            return pts

        def c_py_part(n, p, pts):
            # bare A*V pass for the final combo
            py = ps_py.tile([128, 1024], fp32, name="pspy")
            for st in range(SKT):
                py_stage(p, py, pts, st)
            py_finish(n, p, py)

        def K_(p, off):
            return lambda: ktr_chunk(p, off, 384)

        def B_(q, p):
            return lambda: b_group(q, p)

        def V_(st):
            return lambda: v_tile(st)

        def D_(m, no):
            return lambda: d_group(m, no)

        def O_(p):
            def fn():
                for hh in range(2):
                    for st in range(SKT):
                        o = p * 2304 + st * 256 + hh * 128
                        nc.vector.tensor_scalar_mul(
                            VP[:, o:o + 64], ones[:], maskt[:, st:st + 1])
            return fn

        with nc.allow_low_precision(reason="bf16 within tolerance"):
            # minimal head: K^T(pair2, first key third) + Q^T(block0,
            # pair2), then the exp pipeline starts; everything else —
            # including the rest of K^T(pair2) — is hosted inside pe-parts.
            ktr_chunk(2, 0, 384)
            b_group(0, 2)

            # Global software pipeline: pe-part(i+1) is emitted before
            # py-part(i), so the Scalar engine always has a full combo of
            # exp work queued while PE retires the previous combo's A*V
            # matmuls plus hosted projection groups.
            # NOTE: Tile derives dependencies from EMISSION order, so every
            # producer (V/K/B/D input) must be emitted before its first
            # reader: V tiles inside pe0/pe1 (before py0); K^T(p) and
            # Q^T(n,p) before pe-part(n,p); D(block b) after py-part(b, p1).
            sched = [
                ((0, 2), {0: [K_(2, 384)], 1: [K_(2, 768)], 2: [V_(0)],
                          3: [V_(1)], 4: [V_(2)], 5: [V_(3), O_(2)],
                          6: [V_(4)], 7: [B_(1, 2)]}),
                ((1, 2), {0: [V_(5)], 1: [V_(6)], 2: [V_(7)], 3: [V_(8)],
                          5: [B_(2, 2)]}),
                ((2, 2), {0: [B_(3, 2)], 2: [K_(3, 0)], 4: [O_(3)]}),
                ((3, 2), {0: [K_(3, 384)], 2: [K_(3, 768)],
                          4: [B_(0, 3)]}),
                ((0, 3), {0: [B_(1, 3)], 2: [K_(0, 0)], 4: [O_(0)]}),
                ((1, 3), {0: [B_(2, 3)], 2: [K_(0, 384)], 4: [O_(1)]}),
                ((2, 3), {0: [B_(3, 3)], 2: [K_(0, 768)],
                          4: [K_(1, 0)]}),
                ((3, 3), {0: [K_(1, 384)], 2: [K_(1, 768)],
                          4: [B_(0, 0)]}),
                ((0, 0), {0: [B_(0, 1)], 2: [B_(1, 0)]}),
                ((0, 1), {0: [B_(1, 1)]}),
                ((1, 0), {0: [B_(2, 0)]}),
                ((1, 1), {0: [B_(2, 1)], 2: [D_(0, 0)], 4: [D_(0, 1)],
                          6: [D_(1, 0)], 8: [D_(1, 1)]}),
                ((2, 0), {0: [B_(3, 0)], 2: [D_(2, 0)], 4: [D_(2, 1)],
                          6: [D_(3, 0)], 8: [D_(3, 1)]}),
                ((2, 1), {0: [B_(3, 1)], 2: [D_(4, 0)], 4: [D_(4, 1)],
                          6: [D_(5, 0)], 8: [D_(5, 1)]}),
                ((3, 0), {0: [D_(6, 0)], 2: [D_(6, 1)], 4: [D_(7, 0)],
                          6: [D_(7, 1)]}),
                ((3, 1), {0: [D_(8, 0)], 1: [D_(8, 1)], 2: [D_(9, 0)],
                          3: [D_(9, 1)], 4: [D_(10, 0)], 5: [D_(10, 1)],
                          6: [D_(11, 0)], 7: [D_(11, 1)]}),
            ]
            pending = None
            for (n, p), hosts in sched:
                pts = c_part(n, p, hosts, prev=pending)
                pending = (n, p, pts)
            c_py_part(*pending)
            for m in range(12, 16):
                for no in range(2):
                    d_group(m, no)

    nc.finalize()
    return nc


def _get_program():
    global _PROG
    if _PROG is None:
        _PROG = _build_program()
    return _PROG


def _make_in_maps(inputs):
    from ml_dtypes import bfloat16
    X_Q = np.asarray(inputs["X_Q"], dtype=np.float32)
    X_KV = np.asarray(inputs["X_KV"], dtype=np.float32)
    mask = np.asarray(inputs["key_padding_mask"])
    W_Q = np.asarray(inputs["W_Q"], dtype=np.float32)
    W_K = np.asarray(inputs["W_K"], dtype=np.float32)
    W_V = np.asarray(inputs["W_V"], dtype=np.float32)
    W_O = np.asarray(inputs["W_O"], dtype=np.float32)
    in_maps = []
    for core in range(8):
        b, half = core // 2, core % 2
        idx = np.flatnonzero(~mask[b].astype(bool))
        nk = len(idx)
        assert nk <= SK, f"kept keys {nk} exceed padded SK={SK}"
        xkvc = np.zeros((SK, HID), dtype=np.float32)
        xkvc[:nk] = X_KV[b][idx]
        maskv = (np.arange(SK) < nk).astype(np.float32)

        def wimg(W):
            # c-major image (moving operand): col = c*512 + m
            return (W[:, half * 512:(half + 1) * 512]
                    .reshape(8, 128, 512).transpose(1, 0, 2).reshape(128, 4096))

        def wimg_pair(W):
            # pair-major image (stationary per (pair, c)): col =
            # pair*1024 + c*128 + j, value W[c*128 + p, half*512 + pair*128 + j]
            return (W[:, half * 512:(half + 1) * 512]
                    .reshape(8, 128, 4, 128).transpose(1, 2, 0, 3)
                    .reshape(128, 4096))

        blob = np.zeros((128, BLOBW), dtype=bfloat16)
        blob[:, OWK:OWK + 4096] = wimg_pair(W_K)
        blob[:, OXKV:OXKV + SK * 8] = \
            xkvc.reshape(SK, 8, 128).transpose(2, 1, 0).reshape(128, SK * 8)
        blob[:, OWV:OWV + 4096] = wimg(W_V)
        blob[:, OWQ:OWQ + 4096] = wimg_pair(W_Q)
        blob[:, OXQ:OXQ + 16384] = \
            X_Q[b].reshape(4, 512, 8, 128).transpose(3, 0, 2, 1).reshape(128, 16384)
        blob[:, OWO:OWO + 4096] = \
            (W_O[half * 512:(half + 1) * 512]
             .reshape(4, 128, 1024).transpose(1, 0, 2).reshape(128, 4096))
        blob[:, OMSK:OMSK + SKT] = maskv.reshape(SKT, 128).T
        blob[:, OVPM:OVPM + 576] = np.repeat(
            maskv.reshape(SKT, 128).T[:, :, None], 64, axis=2
        ).reshape(128, SKT * 64)
        in_maps.append({"blob": blob})
    return in_maps


def kernel(**inputs):
    nc = _get_program()
    in_maps = _make_in_maps(inputs)
    res = run_bass_kernel_spmd(nc, in_maps, core_ids=list(range(8)))
    out = np.empty((B, S, HID), dtype=np.float32)
    for b in range(B):
        out[b] = res.results[2 * b]["y"] + res.results[2 * b + 1]["y"]
    return out


# revision 36
# speedup vs baseline: 312.4314x; 1.0139x over previous
import sys

sys.path.insert(0, "/opt/trn_rl_repo")
import numpy as np
from contextlib import ExitStack

from concourse import bacc
import concourse.tile as tile
from concourse import mybir
from concourse.bass_utils import run_bass_kernel_spmd

fp32 = mybir.dt.float32
bf16 = mybir.dt.bfloat16
Exp = mybir.ActivationFunctionType.Exp

B, S, HID = 4, 2048, 1024
H, DK = 16, 64
SK = 1152          # compacted+padded key count (max kept keys = 1036)
SKT = SK // 128    # 9 sk tiles
NPAIR = 4          # head pairs per core (8 heads = half the 16)

# blob column offsets (bf16 words per partition), in DMA/first-use order
OWK = 0
OXKV = OWK + 4096
OWV = OXKV + SK * 8
OWQ = OWV + 4096
OXQ = OWQ + 4096
OWO = OXQ + 16384
OMSK = OWO + 4096
OVPM = OMSK + 16    # masked-ones image for VP denominator columns
BLOBW = OVPM + 576

_PROG = None


def _build_program():
    nc = bacc.Bacc("TRN2", target_bir_lowering=False)

    blob = nc.dram_tensor("blob", [128, BLOBW], bf16, kind="ExternalInput")
    y = nc.dram_tensor("y", [S, HID], fp32, kind="ExternalOutput")

    # SBUF arena (bf16 word offsets per partition), everything resident:
    #   KT   [0..4608)       K^T pair-major: KT[p, pair*1152 + sk]
    #   QT   [4608..12800)   Q^T: QT[p, pair*2048 + sq]
    #   YPN  [12800..20992)  normalized attn out^T: YPN[p, pair*2048 + sq]
    #   VP   [20992..30208)  pair*2304 + st*256 + [Va(64)|ma(64)|Vb(64)|mb(64)]
    #   WK   [30208..34304)  c-major weight chunks
    #   XKV  [34304..43520)  c-major: [:, c*1152 + sk]
    #   WV   [43520..47616)
    #   WQ   [47616..51712)
    #   XQ   [51712..68096)  q-major quarters, c-major within: q*4096 + c*512
    #   WO   [68096..72192)
    arena = nc.alloc_sbuf_tensor("arena", [128, 72192], bf16)
    base = nc.lookup_mloc(arena).addr

    def at(name, words, off_words):
        return nc.alloc_sbuf_tensor_at(
            name, [128, words], bf16, offset=base + off_words * 2
        )

    KT = at("KT", 4608, 0)
    QT = at("QT", 8192, 4608)
    YPN = at("YPN", 8192, 12800)
    VP = at("VP", 9216, 20992)
    WKs = at("WKs", 4096, 30208)
    XKVs = at("XKVs", 9216, 34304)
    WVs = at("WVs", 4096, 43520)
    WQs = at("WQs", 4096, 47616)
    XQs = at("XQs", 16384, 51712)
    WOs = at("WOs", 4096, 68096)

    with tile.TileContext(nc) as tc, ExitStack() as ctx:
        misc = ctx.enter_context(tc.tile_pool(name="misc", bufs=1))
        # pe-part(i+1) fills its 9 exp tiles while py-part(i) still holds
        # its 9, so the exp-tile pool must cover two full combos.
        pt_pool = ctx.enter_context(tc.tile_pool(name="ptp", bufs=18))
        ev_pool = ctx.enter_context(tc.tile_pool(name="evp", bufs=2))
        rc_pool = ctx.enter_context(tc.tile_pool(name="rcp", bufs=1))
        # PSUM bank budget (8 banks total):
        #   ps_e  2 bufs x [128,1024] fp32 = 4 banks   (E tiles + A-phase pv)
        #   ps_py 1 buf  x [128,1024] fp32 = 2 banks   (A*V accumulators, both heads)
        #   ps_h  2 bufs x [128, 512] fp32 = 2 banks   (proj groups: ktr/B/D hosts)
        ps_e = ctx.enter_context(tc.tile_pool(name="pse", bufs=2, space="PSUM"))
        ps_py = ctx.enter_context(tc.tile_pool(name="pspy", bufs=1, space="PSUM"))
        ps_h = ctx.enter_context(tc.tile_pool(name="psh", bufs=2, space="PSUM"))

        masktb = misc.tile([128, 16], bf16)
        nc.sync.dma_start(masktb[:], blob[:, OMSK:OMSK + 16])
        maskt = misc.tile([128, 16], fp32)
        nc.vector.tensor_copy(maskt[:], masktb[:])

        # input stream from the blob, sliced by first use and split across
        # the sync + GPSIMD HWDGE rings. The Scalar engine issues NO DMA
        # descriptors: every dma_start occupies ~0.6us of its in-order
        # queue, which would push the first exp out by ~20us. Weight images
        # are PAIR-major (pair*1024 + c*128), so only the pair-2 columns of
        # W_K/W_Q ride the critical path; X_KV moves as per-c chunks
        # (2.3KB/partition lines, 4 per ring).
        nc.sync.dma_start(WKs[:, 2048:3072], blob[:, OWK + 2048:OWK + 3072])
        nc.gpsimd.dma_start(WQs[:, 2048:3072],
                            blob[:, OWQ + 2048:OWQ + 3072])
        nc.gpsimd.dma_start(XQs[:, 0:4096], blob[:, OXQ:OXQ + 4096])
        for c in range(4):
            nc.sync.dma_start(
                XKVs[:, c * SK:(c + 1) * SK],
                blob[:, OXKV + c * SK: OXKV + (c + 1) * SK])
        for c in range(4, 8):
            nc.gpsimd.dma_start(
                XKVs[:, c * SK:(c + 1) * SK],
                blob[:, OXKV + c * SK: OXKV + (c + 1) * SK])
        nc.gpsimd.dma_start(WVs[:], blob[:, OWV:OWV + 4096])
        nc.gpsimd.dma_start(XQs[:, 4096:8192],
                            blob[:, OXQ + 4096:OXQ + 8192])
        nc.sync.dma_start(WKs[:, 0:2048], blob[:, OWK:OWK + 2048])
        nc.sync.dma_start(WKs[:, 3072:4096], blob[:, OWK + 3072:OWK + 4096])
        nc.sync.dma_start(WQs[:, 0:2048], blob[:, OWQ:OWQ + 2048])
        nc.sync.dma_start(WQs[:, 3072:4096], blob[:, OWQ + 3072:OWQ + 4096])
        nc.sync.dma_start(XQs[:, 8192:12288],
                          blob[:, OXQ + 8192:OXQ + 12288])
        nc.gpsimd.dma_start(XQs[:, 12288:16384],
                            blob[:, OXQ + 12288:OXQ + 16384])
        nc.gpsimd.dma_start(WOs[:], blob[:, OWO:OWO + 4096])

        ones = misc.tile([128, 64], bf16)
        nc.vector.memset(ones[:], 1.0)

        # ---- Phase A: K^T -> KT, V (masked) -> VP ----
        def ktr_chunk(p, off, w):
            pk = ps_h.tile([128, 512], fp32, name="psh")
            for c in range(8):
                nc.tensor.matmul(
                    pk[:, 0:w],
                    WKs[:, p * 1024 + c * 128: p * 1024 + (c + 1) * 128],
                    XKVs[:, c * SK + off: c * SK + off + w],
                    start=(c == 0), stop=(c == 7))
            nc.vector.tensor_copy(
                KT[:, p * SK + off: p * SK + off + w], pk[:, 0:w])

        def v_tile(st):
            pv = ps_e.tile([128, 1024], fp32, name="pe")
            for c in range(8):
                nc.tensor.matmul(
                    pv[:, 0:512],
                    XKVs[:, c * SK + st * 128: c * SK + (st + 1) * 128],
                    WVs[:, c * 512:(c + 1) * 512],
                    start=(c == 0), stop=(c == 7))
            for h in range(8):
                o = (h // 2) * 2304 + st * 256 + (h % 2) * 128 + 64
                nc.vector.tensor_scalar_mul(
                    VP[:, o:o + 64], pv[:, h * 64:(h + 1) * 64],
                    maskt[:, st:st + 1])

        # ---- Q^T projection group ----
        def b_group(q, p):
            pq = ps_h.tile([128, 512], fp32, name="psh")
            for c in range(8):
                nc.tensor.matmul(
                    pq[:],
                    WQs[:, p * 1024 + c * 128: p * 1024 + (c + 1) * 128],
                    XQs[:, q * 4096 + c * 512: q * 4096 + (c + 1) * 512],
                    start=(c == 0), stop=(c == 7))
            nc.vector.tensor_copy(
                QT[:, p * 2048 + q * 512: p * 2048 + (q + 1) * 512], pq[:])

        # VP mask columns (denominator ones, masked, in the LOW 64 cols of
        # each 128 block so the denominator lands in psum partitions 0:64
        # where reciprocal_approx_fast works) are initialized per pair by
        # O_() hosts inside early pe-parts — after the critical K^T/Q^T
        # copies in the DVE queue, before the pair's first py-part.

        # ---- Phases C (attention) and D (out-proj), B/D hosted inside C ----
        def d_group(m, no):
            pd = ps_h.tile([128, 512], fp32, name="psh")
            for tt in range(NPAIR):
                nc.tensor.matmul(
                    pd[:],
                    YPN[:, tt * 2048 + m * 128: tt * 2048 + (m + 1) * 128],
                    WOs[:, tt * 1024 + no * 512: tt * 1024 + no * 512 + 512],
                    start=(tt == 0), stop=(tt == 3))
            ob = ev_pool.tile([128, 512], fp32)
            nc.vector.tensor_copy(ob[:], pd[:])
            nc.sync.dma_start(
                y[m * 128:(m + 1) * 128, no * 512: no * 512 + 512], ob[:])

        def py_finish(n, p, py):
            rc = rc_pool.tile([64, 1024], fp32)
            nc.vector.reciprocal_approx_fast(rc[:], py[0:64, :])
            for hh in (0, 1):
                nc.vector.tensor_mul(
                    YPN[hh * 64:(hh + 1) * 64,
                        p * 2048 + n * 512: p * 2048 + n * 512 + 512],
                    py[64:128, hh * 512:(hh + 1) * 512],
                    rc[:, hh * 512:(hh + 1) * 512])

        def py_stage(p, py, pts, st):
            for hh in (0, 1):
                nc.tensor.matmul(
                    py[:, hh * 512:(hh + 1) * 512],
                    VP[:, p * 2304 + st * 256 + hh * 128:
                       p * 2304 + st * 256 + hh * 128 + 128],
                    pts[st][:, hh * 512:(hh + 1) * 512],
                    start=(st == 0), stop=(st == SKT - 1))

        def c_part(n, p, hosts=None, prev=None):
            # E + exp pass for both heads (hh=0,1) of pair p, query block n.
            # The two E matmuls of each key tile go to row groups 0 and 64
            # of the PE array (tile_position) and run CONCURRENTLY; exp runs
            # one key tile behind, pacing the whole kernel. The PREVIOUS
            # combo's A*V matmuls are interleaved stage-by-stage (a block of
            # 18 of them between combos starves the exp queue), and `hosts`
            # (projection filler groups, keyed by stage) ride in the
            # remaining PE slack. Returns the exp tiles for the next pass.
            qsl = [QT[hh * 64: (hh + 1) * 64,
                      p * 2048 + n * 512: p * 2048 + n * 512 + 512]
                   for hh in (0, 1)]
            pts = []
            ppy = None
            if prev is not None:
                pn, pp, ppts = prev
                ppy = ps_py.tile([128, 1024], fp32, name="pspy")
            for st in range(SKT):
                pe = ps_e.tile([128, 1024], fp32, name="pe")
                for hh in (0, 1):
                    nc.tensor.matmul(
                        pe[:, hh * 512:(hh + 1) * 512],
                        KT[hh * 64:(hh + 1) * 64,
                           p * SK + st * 128: p * SK + (st + 1) * 128],
                        qsl[hh], start=True, stop=True,
                        tile_position=(hh * 64, 0))
                pt = pt_pool.tile([128, 1024], bf16)
                nc.scalar.activation(pt[:], pe[:], Exp, scale=0.125)
                pts.append(pt)
                if prev is not None:
                    py_stage(pp, ppy, ppts, st)
                if hosts and st in hosts:
                    for fn in hosts[st]:
                        fn()
            if prev is not None:
                py_finish(pn, pp, ppy)
            return pts

        def c_py_part(n, p, pts):
            # bare A*V pass for the final combo
            py = ps_py.tile([128, 1024], fp32, name="pspy")
            for st in range(SKT):
                py_stage(p, py, pts, st)
            py_finish(n, p, py)

        def K_(p, off):
            return lambda: ktr_chunk(p, off, 384)

        def B_(q, p):
            return lambda: b_group(q, p)

        def V_(st):
            return lambda: v_tile(st)

        def D_(m, no):
            return lambda: d_group(m, no)

        def O_(p):
            def fn():
                for hh in range(2):
                    for st in range(SKT):
                        o = p * 2304 + st * 256 + hh * 128
                        nc.vector.tensor_scalar_mul(
                            VP[:, o:o + 64], ones[:], maskt[:, st:st + 1])
            return fn

        with nc.allow_low_precision(reason="bf16 within tolerance"):
            # minimal head: K^T(pair2, first key third) + Q^T(block0,
            # pair2), then the exp pipeline starts; everything else —
            # including the rest of K^T(pair2) — is hosted inside pe-parts.
            ktr_chunk(2, 0, 384)
            b_group(0, 2)

            # Global software pipeline: pe-part(i+1) is emitted before
            # py-part(i), so the Scalar engine always has a full combo of
            # exp work queued while PE retires the previous combo's A*V
            # matmuls plus hosted projection groups.
            # NOTE: Tile derives dependencies from EMISSION order, so every
            # producer (V/K/B/D input) must be emitted before its first
            # reader: V tiles inside pe0/pe1 (before py0); K^T(p) and
            # Q^T(n,p) before pe-part(n,p); D(block b) after py-part(b, p1).
            sched = [
                ((0, 2), {0: [K_(2, 384)], 1: [K_(2, 768)], 2: [V_(0)],
                          3: [V_(1)], 4: [V_(2)], 5: [V_(3), O_(2)],
                          6: [V_(4)], 7: [B_(1, 2)]}),
                ((1, 2), {0: [V_(5)], 1: [V_(6)], 2: [V_(7)], 3: [V_(8)],
                          5: [B_(2, 2)]}),
                ((2, 2), {0: [B_(3, 2)], 2: [K_(3, 0)], 4: [O_(3)]}),
                ((3, 2), {0: [K_(3, 384)], 2: [K_(3, 768)],
                          4: [B_(0, 3)]}),
                ((0, 3), {0: [B_(1, 3)], 2: [K_(0, 0)], 4: [O_(0)]}),
                ((1, 3), {0: [B_(2, 3)], 2: [K_(0, 384)], 4: [O_(1)]}),
                ((2, 3), {0: [B_(3, 3)], 2: [K_(0, 768)],
                          4: [K_(1, 0)]}),
                ((3, 3), {0: [K_(1, 384)], 2: [K_(1, 768)],
                          4: [B_(0, 0)]}),
                ((0, 0), {0: [B_(0, 1)], 2: [B_(1, 0)]}),
                ((0, 1), {0: [B_(1, 1)]}),
                ((1, 0), {0: [B_(2, 0)]}),
                ((1, 1), {0: [B_(2, 1)], 2: [D_(0, 0)], 4: [D_(0, 1)],
                          6: [D_(1, 0)], 8: [D_(1, 1)]}),
                ((2, 0), {0: [B_(3, 0)], 2: [D_(2, 0)], 4: [D_(2, 1)],
                          6: [D_(3, 0)], 8: [D_(3, 1)]}),
                ((2, 1), {0: [B_(3, 1)], 2: [D_(4, 0)], 4: [D_(4, 1)],
                          6: [D_(5, 0)], 8: [D_(5, 1)]}),
                ((3, 0), {0: [D_(6, 0)], 2: [D_(6, 1)], 4: [D_(7, 0)],
                          6: [D_(7, 1)]}),
                ((3, 1), {0: [D_(8, 0)], 1: [D_(8, 1)], 2: [D_(9, 0)],
                          3: [D_(9, 1)], 4: [D_(10, 0)], 5: [D_(10, 1)],
                          6: [D_(11, 0)], 7: [D_(11, 1)]}),
            ]
            pending = None
            for (n, p), hosts in sched:
                pts = c_part(n, p, hosts, prev=pending)
                pending = (n, p, pts)
            c_py_part(*pending)
            for m in range(12, 16):
                for no in range(2):
                    d_group(m, no)

    nc.finalize()
    return nc


def _get_program():
    global _PROG
    if _PROG is None:
        _PROG = _build_program()
    return _PROG


def _make_in_maps(inputs):
    from ml_dtypes import bfloat16
    X_Q = np.asarray(inputs["X_Q"], dtype=np.float32)
    X_KV = np.asarray(inputs["X_KV"], dtype=np.float32)
    mask = np.asarray(inputs["key_padding_mask"])
    W_Q = np.asarray(inputs["W_Q"], dtype=np.float32)
    W_K = np.asarray(inputs["W_K"], dtype=np.float32)
    W_V = np.asarray(inputs["W_V"], dtype=np.float32)
    W_O = np.asarray(inputs["W_O"], dtype=np.float32)
    in_maps = []
    for core in range(8):
        b, half = core // 2, core % 2
        idx = np.flatnonzero(~mask[b].astype(bool))
        nk = len(idx)
        assert nk <= SK, f"kept keys {nk} exceed padded SK={SK}"
        xkvc = np.zeros((SK, HID), dtype=np.float32)
        xkvc[:nk] = X_KV[b][idx]
        maskv = (np.arange(SK) < nk).astype(np.float32)

        def wimg(W):
            # c-major image (moving operand): col = c*512 + m
            return (W[:, half * 512:(half + 1) * 512]
                    .reshape(8, 128, 512).transpose(1, 0, 2).reshape(128, 4096))

        def wimg_pair(W):
            # pair-major image (stationary per (pair, c)): col =
            # pair*1024 + c*128 + j, value W[c*128 + p, half*512 + pair*128 + j]
            return (W[:, half * 512:(half + 1) * 512]
                    .reshape(8, 128, 4, 128).transpose(1, 2, 0, 3)
                    .reshape(128, 4096))

        blob = np.zeros((128, BLOBW), dtype=bfloat16)
        blob[:, OWK:OWK + 4096] = wimg_pair(W_K)
        blob[:, OXKV:OXKV + SK * 8] = \
            xkvc.reshape(SK, 8, 128).transpose(2, 1, 0).reshape(128, SK * 8)
        blob[:, OWV:OWV + 4096] = wimg(W_V)
        blob[:, OWQ:OWQ + 4096] = wimg_pair(W_Q)
        blob[:, OXQ:OXQ + 16384] = \
            X_Q[b].reshape(4, 512, 8, 128).transpose(3, 0, 2, 1).reshape(128, 16384)
        blob[:, OWO:OWO + 4096] = \
            (W_O[half * 512:(half + 1) * 512]
             .reshape(4, 128, 1024).transpose(1, 0, 2).reshape(128, 4096))
        blob[:, OMSK:OMSK + SKT] = maskv.reshape(SKT, 128).T
        blob[:, OVPM:OVPM + 576] = np.repeat(
            maskv.reshape(SKT, 128).T[:, :, None], 64, axis=2
        ).reshape(128, SKT * 64)
        in_maps.append({"blob": blob})
    return in_maps


def kernel(**inputs):
    nc = _get_program()
    in_maps = _make_in_maps(inputs)
    res = run_bass_kernel_spmd(nc, in_maps, core_ids=list(range(8)))
    out = np.empty((B, S, HID), dtype=np.float32)
    for b in range(B):
        out[b] = res.results[2 * b]["y"] + res.results[2 * b + 1]["y"]
    return out
